# revision 9
# baseline (speedup 1.0000x reference)
"""CrossAttention Trainium2 SPMD kernel (v3, all-bf16 datapath, pipelined).

Sharding: 8 cores = 2 batches x 4 head-groups (2 heads of 64 dims each).
Core i handles batch b=i//4, inner-dim slice [128*g:128*(g+1)], g=i%4.

Host prep: x/context are pre-transposed and cast to bf16 (xT [D, N]), so the
device needs no input transposes.  Weights are per-core sliced and cast to
bf16.  The output-projection bias is added on the host during the partial-sum
combine (host also sums the 4 inner-dim partial results per batch).

Per-core pipeline:
  A. DMA ctxT then xT (bf16); K^T/V^T projections (d-major); V^T is
     PE-transposed into token-major V with a ones column per head (rowsum
     trick; softmax needs no max subtraction at these score scales).
     All psum->sbuf copies run on DVE to keep ACT free for exp.
  B. Attention per n-chunk c of 1024 (Q^T for chunks 2c,2c+1 is projected
     right before chunk c, overlapping attention with the x DMA tail):
     per m-block of 128: S^T = K_blk^T Q per head as two row-tiled matmuls
     (heads at PE tile rows 0/64 execute concurrently); U = exp(S*scale) on
     ACT (psum->sbuf bf16) -- ACT is the critical path at ~1.1us per
     [128,1024] tile; O_un^T/rowsum accumulate in psum [65,1024] over
     m-blocks.  At chunk end the [65,NC] accumulator is evacuated to fp16
     sbuf immediately (frees the psum bank for the next chunk), then
     normalized: DVE reciprocal of the rowsum row, broadcast across 64
     partitions via a DRAM round-trip, DVE multiply -> O^T bf16.
  C. Y_partial = O^T^T @ Wo_slice per 128-token block, ACT psum->sbuf bf16,
     DMA out (bf16 partials; host sums in fp32).
"""
import numpy as np
import ml_dtypes

import concourse.bass as bass
import concourse.tile as tile
from concourse import bacc, mybir
from concourse.bass_utils import run_bass_kernel_spmd
from concourse.masks import make_identity

F32 = mybir.dt.float32
BF16 = mybir.dt.bfloat16
FP16 = mybir.dt.float16
EXP = mybir.ActivationFunctionType.Exp

D = 1024          # model dim
DG = 128          # inner dims per core (2 heads x 64)
DH = 64           # head dim
SCALE = DH ** -0.5
N_CORES = 8
BF = ml_dtypes.bfloat16


def build(N=4096, M=4096):
    nc = bacc.Bacc("TRN2", target_bir_lowering=False, debug=False,
                   num_devices=N_CORES)
    xt = nc.dram_tensor("xt", [D, N], BF16, kind="ExternalInput").ap()
    ct = nc.dram_tensor("ct", [D, M], BF16, kind="ExternalInput").ap()
    wq = nc.dram_tensor("wq", [D, DG], BF16, kind="ExternalInput").ap()
    wk = nc.dram_tensor("wk", [D, DG], BF16, kind="ExternalInput").ap()
    wv = nc.dram_tensor("wv", [D, DG], BF16, kind="ExternalInput").ap()
    wo = nc.dram_tensor("wo", [DG, D], BF16, kind="ExternalInput").ap()
    y = nc.dram_tensor("y", [N, D], BF16, kind="ExternalOutput").ap()

    with tile.TileContext(nc) as tc:
        _kernel(tc, xt, ct, wq, wk, wv, wo, y, N, M)
    nc.compile()
    return nc


def _kernel(tc, xt, ct, wq, wk, wv, wo, y, N, M):
    nc = tc.nc
    NT_X = N // 512
    NT_C = M // 512
    MB = M // 128
    NC = min(1024, N)
    CH = N // NC
    NS = NC // 512

    from contextlib import ExitStack
    with ExitStack() as ctx:
        consts = ctx.enter_context(tc.tile_pool(name="consts", bufs=1))
        big = ctx.enter_context(tc.tile_pool(name="big", bufs=1))
        upool = ctx.enter_context(tc.tile_pool(name="upool", bufs=3))
        vstage = ctx.enter_context(tc.tile_pool(name="vstage", bufs=1))
        rrpool = ctx.enter_context(tc.tile_pool(name="rrpool", bufs=2))
        avsb = ctx.enter_context(tc.tile_pool(name="avsb", bufs=2))

        # --- weights / constants ---
        wq_sb = consts.tile([128, 8, 128], BF16)
        nc.sync.dma_start(out=wq_sb[:], in_=wq.rearrange("(kb p) c -> p kb c", p=128))
        wk_sb = consts.tile([128, 8, 128], BF16)
        nc.sync.dma_start(out=wk_sb[:], in_=wk.rearrange("(kb p) c -> p kb c", p=128))
        wv_sb = consts.tile([128, 8, 128], BF16)
        nc.sync.dma_start(out=wv_sb[:], in_=wv.rearrange("(kb p) c -> p kb c", p=128))
        wo_sb = consts.tile([64, 2, D], BF16)
        nc.sync.dma_start(out=wo_sb[:], in_=wo.rearrange("(h p) d -> p h d", p=64))

        ident = consts.tile([128, 128], F32)
        make_identity(nc, ident)

        # --- persistent activations ---
        ct_sb = big.tile([128, 8, M], BF16)   # ctx^T: [d%128, kb, m]
        xt_sb = big.tile([128, 8, N], BF16)   # x^T
        QT = big.tile([128, N], BF16)         # [2h*64d, n]
        KT = big.tile([128, M], BF16)
        V_sb = big.tile([128, MB, 132], BF16)  # [m%128, mb, (v_h0|1|pad|v_h1|1|pad)]
        OT = [big.tile([64, N], BF16, name=f"OT{h}") for h in range(2)]

        nc.vector.memset(V_sb[:, :, 64:65], 1.0)
        nc.vector.memset(V_sb[:, :, 130:131], 1.0)

        # --- input DMAs: ctx first (K/V gate attention), then x ---
        ct_r = ct.rearrange("(kb p) m -> p kb m", p=128)
        xt_r = xt.rearrange("(kb p) n -> p kb n", p=128)
        for i in range(M // 1024):
            nc.sync.dma_start(out=ct_sb[:, :, i * 1024:(i + 1) * 1024],
                              in_=ct_r[:, :, i * 1024:(i + 1) * 1024])
        for i in range(N // 1024):
            nc.sync.dma_start(out=xt_sb[:, :, i * 1024:(i + 1) * 1024],
                              in_=xt_r[:, :, i * 1024:(i + 1) * 1024])

        # ---------------- phase A: K/V projections ----------------
        with (
            tc.tile_pool(name="pp", bufs=4, space="PSUM") as pp,
            tc.tile_pool(name="tp", bufs=2, space="PSUM") as tp,
        ):
            for ch in range(NT_C):
                sl = slice(ch * 512, (ch + 1) * 512)
                pk = pp.tile([128, 512], F32, tag="pp", name=f"pk{ch}")
                for kb in range(8):
                    nc.tensor.matmul(pk[:], lhsT=wk_sb[:, kb, :],
                                     rhs=ct_sb[:, kb, sl],
                                     start=(kb == 0), stop=(kb == 7))
                nc.vector.tensor_copy(KT[:, sl], pk[:])
                pv = pp.tile([128, 512], F32, tag="pp", name=f"pv{ch}")
                for kb in range(8):
                    nc.tensor.matmul(pv[:], lhsT=wv_sb[:, kb, :],
                                     rhs=ct_sb[:, kb, sl],
                                     start=(kb == 0), stop=(kb == 7))
                vts = vstage.tile([128, 512], F32, tag="vts", name=f"vts{ch}")
                nc.vector.tensor_copy(vts[:], pv[:])
                tpv = tp.tile([128, 4, 128], F32, tag="tp", name=f"tpv{ch}")
                for tb in range(4):
                    nc.tensor.transpose(tpv[:, tb, :],
                                        vts[:, tb * 128:(tb + 1) * 128], ident[:])
                for h in range(2):
                    nc.vector.tensor_copy(
                        V_sb[:, ch * 4:(ch + 1) * 4, 66 * h:66 * h + 64],
                        tpv[:, :, 64 * h:64 * h + 64])

        # ------- phase B: attention (Q projection interleaved per chunk) ----
        with (
            tc.tile_pool(name="spool", bufs=2, space="PSUM") as spool,
            tc.tile_pool(name="avpool", bufs=2, space="PSUM") as avpool,
            tc.tile_pool(name="drp", bufs=2, space="DRAM") as drp,
        ):
            qchunks_per_c = NT_X // CH
            for c in range(CH):
                # project Q for the n-range this chunk reads
                for j in range(qchunks_per_c):
                    ch = c * qchunks_per_c + j
                    sl = slice(ch * 512, (ch + 1) * 512)
                    pq = spool.tile([128, 512], F32, tag="sp", name=f"pq{ch}")
                    for kb in range(8):
                        nc.tensor.matmul(pq[:], lhsT=wq_sb[:, kb, :],
                                         rhs=xt_sb[:, kb, sl],
                                         start=(kb == 0), stop=(kb == 7))
                    nc.vector.tensor_copy(QT[:, sl], pq[:])

                av = [avpool.tile([65, NC], F32, tag="av", name=f"av{c}_{h}")
                      for h in range(2)]
                for mb in range(MB):
                    sp = [spool.tile([128, NC], F32, tag="sp",
                                     name=f"sp{c}_{mb}_{h}") for h in range(2)]
                    for s in range(NS):
                        for h in range(2):
                            nc.tensor.matmul(
                                sp[h][:, s * 512:(s + 1) * 512],
                                lhsT=KT[64 * h:64 * h + 64,
                                        mb * 128:(mb + 1) * 128],
                                rhs=QT[64 * h:64 * h + 64,
                                       c * NC + s * 512:c * NC + (s + 1) * 512],
                                start=True, stop=True)
                    for h in range(2):
                        u = upool.tile([128, NC], BF16, tag="u",
                                       name=f"u{c}_{mb}_{h}")
                        nc.scalar.activation(u[:], sp[h][:], EXP, scale=SCALE)
                        for s in range(NS):
                            nc.tensor.matmul(
                                av[h][:, s * 512:(s + 1) * 512],
                                lhsT=V_sb[:, mb, 66 * h:66 * h + 65],
                                rhs=u[:, s * 512:(s + 1) * 512],
                                start=(mb == 0), stop=(mb == MB - 1))
                for h in range(2):
                    # evacuate psum accumulator immediately (frees the bank
                    # for the next chunk), then normalize in sbuf
                    avs = avsb.tile([65, NC], FP16, tag="avs",
                                    name=f"avs{c}_{h}")
                    with nc.allow_low_precision(reason="softmax sums fp16"):
                        nc.vector.tensor_copy(avs[:], av[h][:])
                    rr16 = rrpool.tile([1, NC], FP16, tag="rr16",
                                       name=f"rr16{c}_{h}")
                    with nc.allow_low_precision(reason="softmax 1/sum fp16"):
                        nc.vector.reciprocal(rr16[:], avs[64:65, :])
                    rd = drp.tile([NC], FP16, tag="rd", name=f"rd{c}_{h}")
                    nc.sync.dma_start(out=rd[:], in_=rr16[:])
                    rrs = rrpool.tile([64, NC], FP16, tag="rrs", bufs=1,
                                      name=f"rrs{c}_{h}")
                    nc.sync.dma_start(
                        out=rrs[:],
                        in_=bass.AP(tensor=rd.tensor, offset=rd.offset,
                                    ap=[[0, 64]] + list(rd.ap)))
                    nc.vector.tensor_mul(OT[h][:, c * NC:(c + 1) * NC],
                                         avs[0:64, :], rrs[:])

        # ---------------- phase C: output projection ----------------
        with (
            tc.tile_pool(name="ypool", bufs=3, space="PSUM") as ypool,
            tc.tile_pool(name="ysb", bufs=2) as ysb,
        ):
            for nb in range(N // 128):
                yp = ypool.tile([128, D], F32, tag="yp", name=f"yp{nb}")
                for s in range(2):
                    for h in range(2):
                        nc.tensor.matmul(
                            yp[:, s * 512:(s + 1) * 512],
                            lhsT=OT[h][:, nb * 128:(nb + 1) * 128],
                            rhs=wo_sb[:, h, s * 512:(s + 1) * 512],
                            start=(h == 0), stop=(h == 1))
                ys = ysb.tile([128, D], BF16, tag="ys", name=f"ys{nb}")
                nc.scalar.copy(ys[:], yp[:])
                nc.sync.dma_start(out=y[nb * 128:(nb + 1) * 128, :], in_=ys[:])


# ---------------------------------------------------------------------------
_NC_CACHE = {}


def _get_nc():
    if "full" not in _NC_CACHE:
        _NC_CACHE["full"] = build(4096, 4096)
    return _NC_CACHE["full"]


def make_in_maps(x, context, Wq, Wk, Wv, Wo, bo):
    x = np.asarray(x, dtype=np.float32)
    context = np.asarray(context, dtype=np.float32)
    xts = [np.ascontiguousarray(x[b].T).astype(BF) for b in range(2)]
    cts = [np.ascontiguousarray(context[b].T).astype(BF) for b in range(2)]
    Wq = np.asarray(Wq, dtype=np.float32)
    Wk = np.asarray(Wk, dtype=np.float32)
    Wv = np.asarray(Wv, dtype=np.float32)
    Wo = np.asarray(Wo, dtype=np.float32)
    in_maps = []
    for core in range(N_CORES):
        b, g = core // 4, core % 4
        sl = slice(g * DG, (g + 1) * DG)
        in_maps.append({
            "xt": xts[b],
            "ct": cts[b],
            "wq": np.ascontiguousarray(Wq[:, sl]).astype(BF),
            "wk": np.ascontiguousarray(Wk[:, sl]).astype(BF),
            "wv": np.ascontiguousarray(Wv[:, sl]).astype(BF),
            "wo": np.ascontiguousarray(Wo[sl, :]).astype(BF),
        })
    return in_maps


def combine(results, bo):
    bo = np.asarray(bo, dtype=np.float32)
    out = np.empty((2, 4096, 1024), np.float32)
    for b in range(2):
        acc = results[4 * b]["y"].astype(np.float32)
        for g in range(1, 4):
            acc += results[4 * b + g]["y"].astype(np.float32)
        out[b] = acc + bo
    return out


def kernel(x, context, Wq, Wk, Wv, Wo, bo):
    nc = _get_nc()
    in_maps = make_in_maps(x, context, Wq, Wk, Wv, Wo, bo)
    res = run_bass_kernel_spmd(nc, in_maps, list(range(N_CORES))).results
    return combine(res, bo)


# revision 11
# speedup vs baseline: 1.5798x; 1.5798x over previous
"""CrossAttention Trainium2 SPMD kernel (v3, all-bf16 datapath, pipelined).

Sharding: 8 cores = 2 batches x 4 head-groups (2 heads of 64 dims each).
Core i handles batch b=i//4, inner-dim slice [128*g:128*(g+1)], g=i%4.

Host prep: x/context are pre-transposed and cast to bf16 (xT [D, N]), so the
device needs no input transposes.  Weights are per-core sliced and cast to
bf16.  The output-projection bias is added on the host during the partial-sum
combine (host also sums the 4 inner-dim partial results per batch).

Per-core pipeline:
  A. DMA ctxT then xT (bf16); K^T/V^T projections (d-major); V^T is
     PE-transposed into token-major V with a ones column per head (rowsum
     trick; softmax needs no max subtraction at these score scales).
     All psum->sbuf copies run on DVE to keep ACT free for exp.
  B. Attention per n-chunk c of 1024 (Q^T for chunks 2c,2c+1 is projected
     right before chunk c, overlapping attention with the x DMA tail):
     per m-block of 128: S^T = K_blk^T Q per head as two row-tiled matmuls
     (heads at PE tile rows 0/64 execute concurrently); U = exp(S*scale) on
     ACT (psum->sbuf bf16) -- ACT is the critical path at ~1.1us per
     [128,1024] tile; O_un^T/rowsum accumulate in psum [65,1024] over
     m-blocks.  At chunk end the [65,NC] accumulator is evacuated to fp16
     sbuf immediately (frees the psum bank for the next chunk), then
     normalized: DVE reciprocal of the rowsum row, broadcast across 64
     partitions via a DRAM round-trip, DVE multiply -> O^T bf16.
  C. Y_partial = O^T^T @ Wo_slice per 128-token block, ACT psum->sbuf bf16,
     DMA out (bf16 partials; host sums in fp32).
"""
import numpy as np
import ml_dtypes

import concourse.bass as bass
import concourse.tile as tile
from concourse import bacc, mybir
from concourse.bass_utils import run_bass_kernel_spmd
from concourse.masks import make_identity

F32 = mybir.dt.float32
BF16 = mybir.dt.bfloat16
FP16 = mybir.dt.float16
EXP = mybir.ActivationFunctionType.Exp

D = 1024          # model dim
DG = 128          # inner dims per core (2 heads x 64)
DH = 64           # head dim
SCALE = DH ** -0.5
N_CORES = 8
BF = ml_dtypes.bfloat16


def build(N=4096, M=4096):
    nc = bacc.Bacc("TRN2", target_bir_lowering=False, debug=False,
                   num_devices=N_CORES)
    xt = nc.dram_tensor("xt", [D, N], BF16, kind="ExternalInput").ap()
    ct = nc.dram_tensor("ct", [D, M], BF16, kind="ExternalInput").ap()
    wq = nc.dram_tensor("wq", [D, DG], BF16, kind="ExternalInput").ap()
    wk = nc.dram_tensor("wk", [D, DG], BF16, kind="ExternalInput").ap()
    wv = nc.dram_tensor("wv", [D, DG], BF16, kind="ExternalInput").ap()
    wo = nc.dram_tensor("wo", [DG, D], BF16, kind="ExternalInput").ap()
    y = nc.dram_tensor("y", [N, D], BF16, kind="ExternalOutput").ap()

    with tile.TileContext(nc) as tc:
        _kernel(tc, xt, ct, wq, wk, wv, wo, y, N, M)
    nc.compile()
    return nc


def _kernel(tc, xt, ct, wq, wk, wv, wo, y, N, M):
    nc = tc.nc
    NT_X = N // 512
    NT_C = M // 512
    MB = M // 128
    NC = min(1024, N)
    CH = N // NC
    NS = NC // 512

    from contextlib import ExitStack
    with ExitStack() as ctx:
        consts = ctx.enter_context(tc.tile_pool(name="consts", bufs=1))
        big = ctx.enter_context(tc.tile_pool(name="big", bufs=1))
        upool = ctx.enter_context(tc.tile_pool(name="upool", bufs=3))
        vstage = ctx.enter_context(tc.tile_pool(name="vstage", bufs=1))
        rrpool = ctx.enter_context(tc.tile_pool(name="rrpool", bufs=2))
        avsb = ctx.enter_context(tc.tile_pool(name="avsb", bufs=2))

        # --- weights / constants ---
        wq_sb = consts.tile([128, 8, 128], BF16)
        nc.sync.dma_start(out=wq_sb[:], in_=wq.rearrange("(kb p) c -> p kb c", p=128))
        wk_sb = consts.tile([128, 8, 128], BF16)
        nc.sync.dma_start(out=wk_sb[:], in_=wk.rearrange("(kb p) c -> p kb c", p=128))
        wv_sb = consts.tile([128, 8, 128], BF16)
        nc.sync.dma_start(out=wv_sb[:], in_=wv.rearrange("(kb p) c -> p kb c", p=128))
        wo_sb = consts.tile([64, 2, D], BF16)
        nc.sync.dma_start(out=wo_sb[:], in_=wo.rearrange("(h p) d -> p h d", p=64))

        ident = consts.tile([128, 128], F32)
        make_identity(nc, ident)

        # --- persistent activations ---
        ct_sb = big.tile([128, 8, M], BF16)   # ctx^T: [d%128, kb, m]
        xt_sb = big.tile([128, 8, N], BF16)   # x^T
        QT = big.tile([128, N], BF16)         # [2h*64d, n]
        KT = big.tile([128, M], BF16)
        V_sb = big.tile([128, MB, 132], BF16)  # [m%128, mb, (v_h0|1|pad|v_h1|1|pad)]
        OT = [big.tile([64, N], BF16, name=f"OT{h}") for h in range(2)]

        nc.vector.memset(V_sb[:, :, 64:65], 1.0)
        nc.vector.memset(V_sb[:, :, 130:131], 1.0)

        # --- input DMAs: ctx first (K/V gate attention), then x ---
        ct_r = ct.rearrange("(kb p) m -> p kb m", p=128)
        xt_r = xt.rearrange("(kb p) n -> p kb n", p=128)
        for i in range(M // 1024):
            nc.sync.dma_start(out=ct_sb[:, :, i * 1024:(i + 1) * 1024],
                              in_=ct_r[:, :, i * 1024:(i + 1) * 1024])
        for i in range(N // 1024):
            nc.sync.dma_start(out=xt_sb[:, :, i * 1024:(i + 1) * 1024],
                              in_=xt_r[:, :, i * 1024:(i + 1) * 1024])

        # ---------------- phase A: K/V projections ----------------
        with (
            tc.tile_pool(name="pp", bufs=4, space="PSUM") as pp,
            tc.tile_pool(name="tp", bufs=2, space="PSUM") as tp,
        ):
            for ch in range(NT_C):
                sl = slice(ch * 512, (ch + 1) * 512)
                pk = pp.tile([128, 512], F32, tag="pp", name=f"pk{ch}")
                for kb in range(8):
                    nc.tensor.matmul(pk[:], lhsT=wk_sb[:, kb, :],
                                     rhs=ct_sb[:, kb, sl],
                                     start=(kb == 0), stop=(kb == 7))
                nc.vector.tensor_copy(KT[:, sl], pk[:])
                pv = pp.tile([128, 512], F32, tag="pp", name=f"pv{ch}")
                for kb in range(8):
                    nc.tensor.matmul(pv[:], lhsT=wv_sb[:, kb, :],
                                     rhs=ct_sb[:, kb, sl],
                                     start=(kb == 0), stop=(kb == 7))
                vts = vstage.tile([128, 512], F32, tag="vts", name=f"vts{ch}")
                nc.vector.tensor_copy(vts[:], pv[:])
                tpv = tp.tile([128, 4, 128], F32, tag="tp", name=f"tpv{ch}")
                for tb in range(4):
                    nc.tensor.transpose(tpv[:, tb, :],
                                        vts[:, tb * 128:(tb + 1) * 128], ident[:])
                for h in range(2):
                    nc.vector.tensor_copy(
                        V_sb[:, ch * 4:(ch + 1) * 4, 66 * h:66 * h + 64],
                        tpv[:, :, 64 * h:64 * h + 64])

        # ------- phase B: attention (Q projection interleaved per chunk) ----
        with (
            tc.tile_pool(name="spool", bufs=2, space="PSUM") as spool,
            tc.tile_pool(name="avpool", bufs=2, space="PSUM") as avpool,
            tc.tile_pool(name="drp", bufs=2, space="DRAM") as drp,
        ):
            qchunks_per_c = NT_X // CH

            def qproj(c):
                for j in range(qchunks_per_c):
                    ch = c * qchunks_per_c + j
                    sl = slice(ch * 512, (ch + 1) * 512)
                    pq = spool.tile([128, 512], F32, tag="sp", name=f"pq{ch}")
                    for kb in range(8):
                        nc.tensor.matmul(pq[:], lhsT=wq_sb[:, kb, :],
                                         rhs=xt_sb[:, kb, sl],
                                         start=(kb == 0), stop=(kb == 7))
                    nc.vector.tensor_copy(QT[:, sl], pq[:])

            def s_pair(c, mb, h, sp_tiles):
                """S^T for (c, mb, h): two 512-col matmuls at PE tile row 64h."""
                for s in range(NS):
                    nc.tensor.matmul(
                        sp_tiles[h][:, s * 512:(s + 1) * 512],
                        lhsT=KT[64 * h:64 * h + 64, mb * 128:(mb + 1) * 128],
                        rhs=QT[64 * h:64 * h + 64,
                               c * NC + s * 512:c * NC + (s + 1) * 512],
                        start=True, stop=True)

            qproj(0)
            for c in range(CH):
                av = [avpool.tile([65, NC], F32, tag="av", name=f"av{c}_{h}")
                      for h in range(2)]
                # prologue: scores for mb=0
                sp = [spool.tile([128, NC], F32, tag="sp",
                                 name=f"sp{c}_0_{h}") for h in range(2)]
                for h in range(2):
                    s_pair(c, 0, h, sp)
                for mb in range(MB):
                    # software pipeline: while ACT runs exp(mb, h), PE has
                    # already been handed AV(mb, h') and the S pair for mb+1
                    # of the slot exp(mb, h') just freed.
                    spn = ([spool.tile([128, NC], F32, tag="sp",
                                       name=f"sp{c}_{mb+1}_{h}")
                            for h in range(2)] if mb + 1 < MB else None)
                    for h in range(2):
                        u = upool.tile([128, NC], BF16, tag="u",
                                       name=f"u{c}_{mb}_{h}")
                        nc.scalar.activation(u[:], sp[h][:], EXP, scale=SCALE)
                        for s in range(NS):
                            nc.tensor.matmul(
                                av[h][:, s * 512:(s + 1) * 512],
                                lhsT=V_sb[:, mb, 66 * h:66 * h + 65],
                                rhs=u[:, s * 512:(s + 1) * 512],
                                start=(mb == 0), stop=(mb == MB - 1))
                        if spn is not None:
                            s_pair(c, mb + 1, h, spn)
                    if mb == MB // 2 and c + 1 < CH:
                        qproj(c + 1)
                    if spn is not None:
                        sp = spn
                for h in range(2):
                    # evacuate psum accumulator immediately (frees the bank
                    # for the next chunk), then normalize in sbuf
                    avs = avsb.tile([65, NC], FP16, tag="avs",
                                    name=f"avs{c}_{h}")
                    with nc.allow_low_precision(reason="softmax sums fp16"):
                        nc.vector.tensor_copy(avs[:], av[h][:])
                    rr16 = rrpool.tile([1, NC], FP16, tag="rr16",
                                       name=f"rr16{c}_{h}")
                    with nc.allow_low_precision(reason="softmax 1/sum fp16"):
                        nc.vector.reciprocal(rr16[:], avs[64:65, :])
                    rd = drp.tile([NC], FP16, tag="rd", name=f"rd{c}_{h}")
                    nc.sync.dma_start(out=rd[:], in_=rr16[:])
                    rrs = rrpool.tile([64, NC], FP16, tag="rrs", bufs=1,
                                      name=f"rrs{c}_{h}")
                    nc.sync.dma_start(
                        out=rrs[:],
                        in_=bass.AP(tensor=rd.tensor, offset=rd.offset,
                                    ap=[[0, 64]] + list(rd.ap)))
                    nc.vector.tensor_mul(OT[h][:, c * NC:(c + 1) * NC],
                                         avs[0:64, :], rrs[:])

        # ---------------- phase C: output projection ----------------
        with (
            tc.tile_pool(name="ypool", bufs=3, space="PSUM") as ypool,
            tc.tile_pool(name="ysb", bufs=2) as ysb,
        ):
            for nb in range(N // 128):
                yp = ypool.tile([128, D], F32, tag="yp", name=f"yp{nb}")
                for s in range(2):
                    for h in range(2):
                        nc.tensor.matmul(
                            yp[:, s * 512:(s + 1) * 512],
                            lhsT=OT[h][:, nb * 128:(nb + 1) * 128],
                            rhs=wo_sb[:, h, s * 512:(s + 1) * 512],
                            start=(h == 0), stop=(h == 1))
                ys = ysb.tile([128, D], BF16, tag="ys", name=f"ys{nb}")
                if nb % 2 == 0:
                    nc.scalar.copy(ys[:], yp[:])
                else:
                    nc.vector.tensor_copy(ys[:], yp[:])
                nc.sync.dma_start(out=y[nb * 128:(nb + 1) * 128, :], in_=ys[:])


# ---------------------------------------------------------------------------
_NC_CACHE = {}


def _get_nc():
    if "full" not in _NC_CACHE:
        _NC_CACHE["full"] = build(4096, 4096)
    return _NC_CACHE["full"]


def make_in_maps(x, context, Wq, Wk, Wv, Wo, bo):
    x = np.asarray(x, dtype=np.float32)
    context = np.asarray(context, dtype=np.float32)
    xts = [np.ascontiguousarray(x[b].T).astype(BF) for b in range(2)]
    cts = [np.ascontiguousarray(context[b].T).astype(BF) for b in range(2)]
    Wq = np.asarray(Wq, dtype=np.float32)
    Wk = np.asarray(Wk, dtype=np.float32)
    Wv = np.asarray(Wv, dtype=np.float32)
    Wo = np.asarray(Wo, dtype=np.float32)
    in_maps = []
    for core in range(N_CORES):
        b, g = core // 4, core % 4
        sl = slice(g * DG, (g + 1) * DG)
        in_maps.append({
            "xt": xts[b],
            "ct": cts[b],
            "wq": np.ascontiguousarray(Wq[:, sl]).astype(BF),
            "wk": np.ascontiguousarray(Wk[:, sl]).astype(BF),
            "wv": np.ascontiguousarray(Wv[:, sl]).astype(BF),
            "wo": np.ascontiguousarray(Wo[sl, :]).astype(BF),
        })
    return in_maps


def combine(results, bo):
    bo = np.asarray(bo, dtype=np.float32)
    out = np.empty((2, 4096, 1024), np.float32)
    for b in range(2):
        acc = results[4 * b]["y"].astype(np.float32)
        for g in range(1, 4):
            acc += results[4 * b + g]["y"].astype(np.float32)
        out[b] = acc + bo
    return out


def kernel(x, context, Wq, Wk, Wv, Wo, bo):
    nc = _get_nc()
    in_maps = make_in_maps(x, context, Wq, Wk, Wv, Wo, bo)
    res = run_bass_kernel_spmd(nc, in_maps, list(range(N_CORES))).results
    return combine(res, bo)
